# revision 24
# baseline (speedup 1.0000x reference)
"""Multi-head attention (16 heads, E=1024, seq=2048, batch=4) on 8 NeuronCores.

Sharding: core = 2*b + g  (b = batch 0..3, g = head-group 0..1, 8 heads each).
Each core computes its batch's QKV for its 8 heads, attention, and a partial
output projection (rows of W_out for its heads); host sums the two partials
per batch and adds b_out.

On-chip layout avoids all transposes:
  - host supplies x^T [1024, 2048] per core (bf16)
  - q^T,k^T computed as (W^T x^T)  -> [qk_col, seq]   (lhsT = W chunk)
  - v computed naturally as x @ W_v -> [seq, v_col]   (lhsT = x^T chunk)
  - scores^T[sk, sq] = (k^T chunk)^T.T @ q^T  (lhsT = k^T slice, rhs = q^T);
    head pairs share one PSUM tile ([A sq512 | B sq512]) with the two
    64-contraction matmuls row-packed via tile_position (they execute
    concurrently on different PE row groups), so one Exp covers both heads
  - softmax denominator via an appended ones-column in the PV lhsT
  - PV: out^T[d(+1), sq] = [v | 1]^T @ attn^T, accumulated over sk chunks
  - normalize: fast-approx reciprocal of the denominator ROW (1x512) on DVE,
    broadcast across partitions with a K=1 matmul, multiply on DVE; the
    broadcast+multiply are deferred into the next sq-block's chunk stream
  - proj: y[sq, :] from lhsT = out^T tiles, rhs = W_out rows for this group

All weights/activations on the matmul paths are bf16 (full PE rate, FWL
weight loads, half DMA) with fp32 PSUM accumulation; the denominator
reciprocal/broadcast path stays fp32/f32r.

Scheduling: phase B (attention) is paced by the ACT Exp stream, so all
other matmul work (remaining V and q/k for pair 0 via need-by-markers,
next pair's q/k, final projection) is dribbled into the chunk streams to
fill PE slack. The prologue is minimal: q(jb0) + k(jb0) + v(chunk0) right
behind the input DMA, then attention starts and the rest streams in.
"""

import sys

sys.path.insert(0, "/opt/trn_rl_repo")

import ml_dtypes
import numpy as np

import concourse.bacc as bacc
import concourse.mybir as mybir
import concourse.tile as tile
from concourse import bass_utils

P = 128
SEQ = 2048
EMB = 1024
N_HEADS_CORE = 8
D_HEAD = 64
QK_COLS = 1024          # q(512) + k(512) for this core's heads
V_COLS = 512
VA = D_HEAD + 1         # v columns per head incl. ones column
N_CORES = 8
NORM = 0.125            # 1/sqrt(64), folded into W_q/b_q on host

F32 = mybir.dt.float32
F32R = mybir.dt.float32r
BF16 = mybir.dt.bfloat16
AF = mybir.ActivationFunctionType
import os
X_DT = {"bf16": BF16, "f32r": F32R}[os.environ.get("K_X", "bf16")]
QK_DT = {"bf16": BF16, "f32r": F32R}[os.environ.get("K_QK", "bf16")]
AT_DT = {"bf16": BF16, "f32r": F32R}[os.environ.get("K_AT", "bf16")]
V_DT = {"bf16": BF16, "f32r": F32R}[os.environ.get("K_V", "bf16")]
OT_DT = {"bf16": BF16, "f32r": F32R}[os.environ.get("K_OT", "bf16")]
HOST_DT_MAP = {"bf16": None, "f32r": None}

KC = EMB // P          # 8 contraction chunks
NSC = SEQ // P         # 16 seq chunks of 128
NJB = SEQ // 512       # 4 sq blocks of 512

_CACHED = None


def _build():
    nc = bacc.Bacc("TRN2", target_bir_lowering=False, debug=False,
                   enable_asserts=True, num_devices=N_CORES)

    xT = nc.dram_tensor("xT", [EMB, SEQ], X_DT, kind="ExternalInput").ap()
    wqk = nc.dram_tensor("wqk", [EMB, QK_COLS], X_DT, kind="ExternalInput").ap()
    wv = nc.dram_tensor("wv", [EMB, V_COLS], X_DT, kind="ExternalInput").ap()
    wo = nc.dram_tensor("wo", [V_COLS, EMB], X_DT, kind="ExternalInput").ap()
    bqk = nc.dram_tensor("bqk", [P, QK_COLS // P], F32, kind="ExternalInput").ap()
    bv = nc.dram_tensor("bv", [1, V_COLS], F32, kind="ExternalInput").ap()
    out = nc.dram_tensor("out", [SEQ, EMB], BF16, kind="ExternalOutput").ap()

    with tile.TileContext(nc) as tc:
      with tc.tile_pool(name="persist", bufs=1) as persist, \
           tc.tile_pool(name="qkT", bufs=2) as qkT_pool, \
           tc.tile_pool(name="oTp", bufs=1) as oT_pool, \
           tc.tile_pool(name="attn", bufs=3) as attn_pool, \
           tc.tile_pool(name="nrm", bufs=2) as nrm_pool, \
           tc.tile_pool(name="ps_s", bufs=2, space="PSUM") as ps_s_pool, \
           tc.tile_pool(name="ps_o0", bufs=1, space="PSUM") as ps_o0_pool, \
           tc.tile_pool(name="ps_o1", bufs=1, space="PSUM") as ps_o1_pool:
        ps_o_pools = [ps_o0_pool, ps_o1_pool]
        vsb = [persist.tile([P, N_HEADS_CORE * VA], V_DT, tag=f"v{s}", name=f"v{s}")
               for s in range(NSC)]
        bqk_sb = persist.tile([P, QK_COLS // P], F32, tag="bqk")
        bv_sb = persist.tile([P, V_COLS], F32, tag="bv")
        nc.sync.dma_start(bqk_sb[:], bqk)
        nc.sync.dma_start(bv_sb[:], bv[0:1, :].broadcast_to([P, V_COLS]))
        ones_sb = persist.tile([P, D_HEAD], F32R, tag="ones")
        nc.vector.tensor_scalar(ones_sb[:], bv_sb[:, 0:D_HEAD], 0.0, 1.0,
                                mybir.AluOpType.mult, mybir.AluOpType.add)
        ones_f32 = persist.tile([P, 512], F32, tag="ones_f32")
        nc.vector.tensor_scalar(ones_f32[:], bv_sb[:, :], 0.0, 1.0,
                                mybir.AluOpType.mult, mybir.AluOpType.add)

        qT = {}
        kT = {}
        outT = [oT_pool.tile([P, SEQ], OT_DT, tag=f"oT{t}", name=f"oT{t}")
                for t in range(4)]

        pending = [None]

        def emit_B_pair(t, fillers, scratch_pool, need=None, after_jb=None,
                        dynamic=False, budget=3, flush=False, blocks=None):
            """Head pair (2t, 2t+1): rows 0-63 / 64-127 of qT[t]/kT[t].
            Per chunk one ps_s [128,1024] = [A sq512 | B sq512]; scores
            row-packed, one exp for both heads, PV splits to per-head
            accumulators. `fillers` are thunks sprinkled into the chunk
            stream to fill PE slack under the ACT-paced exp pipeline.
            `need[i]` (optional) = iteration index before whose scores/PV
            filler i must have been emitted (JIT production for pair 0)."""
            kTh = kT[t]
            qTh = qT[t]
            it = 0
            fi = 0
            nfill = len(fillers)
            if blocks is None:
                blocks = [(j * 512, 512) for j in range(NJB)]
            for j, (sq0, w) in enumerate(blocks):
                ps_os = [ps_o_pools[hh].tile([VA, w], F32, tag=f"ps_o{hh}",
                                             name=f"ps_o{t}_{j}_{hh}")
                         for hh in range(2)]

                def scores(c):
                    ps_s = ps_s_pool.tile([P, 2 * w], F32, tag="ps_s",
                                          name=f"ps_s{t}_{j}_{c}")
                    for hh in range(2):
                        pr = hh * D_HEAD
                        nc.tensor.matmul(
                            ps_s[:, hh * w:(hh + 1) * w],
                            kTh[pr:pr + D_HEAD, c * P:(c + 1) * P],
                            qTh[pr:pr + D_HEAD, sq0:sq0 + w],
                            start=True, stop=True, tile_position=(pr, 0))
                    return ps_s

                ps_s = scores(0)
                for c in range(NSC):
                    at = attn_pool.tile([P, 2 * w], AT_DT, tag="attnT",
                                        name=f"at{t}_{j}_{c}")
                    nc.scalar.activation(at[:], ps_s[:], AF.Exp)
                    if c + 1 < NSC:
                        ps_s = scores(c + 1)
                    if need is not None:
                        while fi < nfill and need[fi] <= it:
                            fillers[fi]()
                            fi += 1
                    va3 = vsb[c][:].rearrange("p (h c) -> p h c", c=VA)
                    for hh in range(2):
                        nc.tensor.matmul(
                            ps_os[hh][:],
                            va3[:, 2 * t + hh, :],
                            at[:, hh * w:(hh + 1) * w],
                            start=(c == 0), stop=(c == NSC - 1))
                    it += 1
                    if c == 10 and pending[0] is not None:
                        fin = pending[0]
                        pending[0] = None
                        fin()
                    if dynamic or need is not None:
                        b = budget
                        while fi < len(fillers) and b > 0:
                            fillers[fi]()
                            fi += 1
                            b -= 1
                    else:
                        while nfill and fi < (nfill * it) // 64 and fi < nfill:
                            fillers[fi]()
                            fi += 1

                # stage 1 (DVE): evacuate ps_o FIRST (frees the PSUM banks
                # for the next jb's PV), then ONE reciprocal over both
                # heads' denominator rows (partition-stacked) + f32r cast
                outUs = []
                for hh in range(2):
                    outU = nrm_pool.tile([VA, w], F32, tag=f"outU{hh}",
                                         name=f"outU{t}_{j}_{hh}", bufs=2)
                    nc.vector.tensor_copy(outU[:], ps_os[hh][:])
                    outUs.append(outU)
                if flush and j == len(blocks) - 1:
                    # final block: ACT is idle after the last exp, so the
                    # reciprocal runs there as exp(-ln(d)) — much shorter
                    # serial chain than the DVE reciprocal
                    rc_rows = []
                    for hh in range(2):
                        rln = nrm_pool.tile([VA, w], F32, tag=f"rln{hh}",
                                            name=f"rln{t}_{j}_{hh}", bufs=2)
                        nc.scalar.activation(rln[D_HEAD:VA, :],
                                             outUs[hh][D_HEAD:VA, :], AF.Ln)
                        rca = nrm_pool.tile([VA, w], F32R, tag=f"rca{hh}",
                                            name=f"rca{t}_{j}_{hh}", bufs=2)
                        nc.scalar.activation(rca[D_HEAD:VA, :],
                                             rln[D_HEAD:VA, :], AF.Exp,
                                             scale=-1.0)
                        rc_rows.append((rca, D_HEAD))
                else:
                    # partition-stacked reciprocal: both heads' denominator
                    # rows in one [33,w] tile (rows 0/32), ONE DVE
                    # reciprocal, deferred off the critical path
                    rden = nrm_pool.tile([33, w], F32, tag="rden",
                                         name=f"rden{t}_{j}", bufs=2)
                    nc.vector.tensor_scalar(rden[:], bv_sb[0:33, 0:w],
                                            0.0, 1.0, mybir.AluOpType.mult,
                                            mybir.AluOpType.add)
                    for hh in range(2):
                        nc.vector.tensor_copy(rden[32 * hh:32 * hh + 1, :],
                                              outUs[hh][D_HEAD:VA, :])
                    rrec = nrm_pool.tile([33, w], F32, tag="rrec",
                                         name=f"rrec{t}_{j}", bufs=2)
                    nc.vector.reciprocal(rrec[:], rden[:])
                    rcast = nrm_pool.tile([33, w], F32R, tag="rcast",
                                          name=f"rcast{t}_{j}", bufs=2)
                    with nc.allow_low_precision(reason="denom cast f32r"):
                        nc.vector.tensor_copy(rcast[:], rrec[:])
                    rc_rows = [(rcast, 0), (rcast, 32)]

                # stage 2 (PE bcast + DVE mul): deferred into the NEXT
                # chunk stream (possibly the next pair's) so the PE never
                # waits on the reciprocal chain
                def make_fin(tt, jj, sq00, ww, oUs, rc, ajb):
                    def fin():
                        psb = ps_s_pool.tile([P, 2 * ww], F32, tag="ps_s",
                                             name=f"psb{tt}_{jj}")
                        for hh in range(2):
                            tile_, row = rc[hh]
                            nc.tensor.matmul(psb[0:D_HEAD,
                                                 hh * ww:(hh + 1) * ww],
                                             ones_sb[row:row + 1, :],
                                             tile_[row:row + 1, :],
                                             start=True, stop=True,
                                             tile_position=(row - row % 32,
                                                            0))
                        for hh in range(2):
                            with nc.allow_low_precision(reason="outT bf16"):
                                nc.vector.tensor_mul(
                                    outT[tt][hh * D_HEAD:(hh + 1) * D_HEAD,
                                             sq00:sq00 + ww],
                                    oUs[hh][0:D_HEAD, :],
                                    psb[0:D_HEAD, hh * ww:(hh + 1) * ww])
                        if ajb is not None:
                            fillers.extend(ajb(sq00, ww))
                    return fin

                pending[0] = make_fin(t, j, sq0, w, outUs, rc_rows, after_jb)
            if flush and pending[0] is not None:
                fin = pending[0]
                pending[0] = None
                fin()
            while fi < len(fillers):
                fillers[fi]()
                fi += 1

        # ---- phase A scaffolding (xT, wqk, wv all loaded upfront) ----
        with tc.tile_pool(name="xTp", bufs=1) as xTp, \
             tc.tile_pool(name="wqkp", bufs=1) as wqkp, \
             tc.tile_pool(name="wvp", bufs=1) as wvp, \
             tc.tile_pool(name="psA", bufs=2, space="PSUM") as psA:
            xT_all = xTp.tile([P, KC * SEQ], X_DT, tag="xT", name="xT_all")
            wqkT_sb = [wqkp.tile([P, KC * P], X_DT, tag=f"wqkT{t}",
                                 name=f"wqkT{t}") for t in range(8)]
            wv_all = wvp.tile([P, KC * V_COLS], X_DT, tag="wv", name="wv_all")
            # wqk arrives host-pretiled as [t p, k c]; DMA the two col
            # tiles the first exp needs before everything else, then x
            # (the 4MB long pole), then wv, then the remaining tiles
            for t in (0, 4):
                nc.sync.dma_start(wqkT_sb[t][:], wqk[t * P:(t + 1) * P, :])
            nc.scalar.dma_start(
                xT_all[:].rearrange("p (k c) -> p k c", k=KC),
                xT[:].rearrange("(k p) c -> p k c", p=P))
            nc.scalar.dma_start(
                wv_all[:].rearrange("p (k c) -> p k c", k=KC),
                wv[:].rearrange("(k p) c -> p k c", p=P))
            for t in (1, 5, 2, 6, 3, 7):
                nc.scalar.dma_start(wqkT_sb[t][:], wqk[t * P:(t + 1) * P, :])
            xT_sb = [xT_all[:, k * SEQ:(k + 1) * SEQ] for k in range(KC)]
            wv_sb = [wv_all[:, k * V_COLS:(k + 1) * V_COLS]
                     for k in range(KC)]

            def make_qk_tile(t):
                if t < 4:
                    qT[t] = qkT_pool.tile([P, SEQ], QK_DT, tag="qTa",
                                          name=f"qT{t}")
                else:
                    kT[t - 4] = qkT_pool.tile([P, SEQ], QK_DT, tag="kTa",
                                              name=f"kT{t-4}")

            def qk_mm(t, j, k, state):
                """One matmul of column tile t (q if t<4 else k), sq/sk
                group j, contraction chunk k; bias-add evacuation at k=7."""
                dst = qT[t] if t < 4 else kT[t - 4]

                def go():
                    if k == 0:
                        state[j] = psA.tile([P, 512], F32, tag="psA_t",
                                            name=f"psqk{t}_{j}")
                    ps = state[j]
                    nc.tensor.matmul(
                        ps[:], wqkT_sb[t][:, k * P:(k + 1) * P],
                        xT_sb[k][:, j * 512:(j + 1) * 512],
                        start=(k == 0), stop=(k == KC - 1))
                    if k == KC - 1:
                        with nc.allow_low_precision(reason="qk bf16"):
                            nc.vector.tensor_scalar_add(
                                dst[:, j * 512:(j + 1) * 512], ps[:],
                                bqk_sb[:, t:t + 1])
                return go

            def v_mm(s, k, state):
                def go():
                    if k == 0:
                        state[s] = psA.tile([P, V_COLS], F32, tag="psA_t",
                                            name=f"psv{s}")
                    ps = state[s]
                    nc.tensor.matmul(
                        ps[:], xT_sb[k][:, s * P:(s + 1) * P], wv_sb[k][:],
                        start=(k == 0), stop=(k == KC - 1))
                    if k == KC - 1:
                        v3 = vsb[s][:].rearrange("p (h c) -> p h c", c=VA)
                        ps3 = ps[:].rearrange("p (h c) -> p h c", c=D_HEAD)
                        bv3 = bv_sb[:].rearrange("p (h c) -> p h c", c=D_HEAD)
                        with nc.allow_low_precision(reason="v bf16"):
                            nc.vector.tensor_add(v3[:, :, 0:D_HEAD], ps3, bv3)
                            nc.vector.tensor_scalar(
                                v3[:, :, D_HEAD], bv_sb[:, 0:N_HEADS_CORE],
                                0.0, 1.0, mybir.AluOpType.mult,
                                mybir.AluOpType.add)
                return go

            # minimal prologue: q jb0, k group0, v chunk0 — then attention
            # starts and everything else dribbles into the chunk stream
            JIT = os.environ.get("K_JIT", "1") == "1"
            make_qk_tile(0)
            make_qk_tile(4)
            st_q0, st_k0, st_v = {}, {}, {}
            if JIT:
                for k in range(KC):
                    qk_mm(0, 0, k, st_q0)()
                    qk_mm(4, 0, k, st_k0)()
            else:
                for j in range(NJB):
                    for k in range(KC):
                        qk_mm(0, j, k, st_q0)()
                for g in range(NJB):
                    for k in range(KC):
                        qk_mm(4, g, k, st_k0)()
                for s in range(NSC):
                    for k in range(KC):
                        v_mm(s, k, st_v)()

            # pair 0 fillers with need-by markers (JIT production):
            #   k^T group g   -> before iteration 4g-1 (scores of chunk 4g)
            #   v chunk s     -> before iteration s    (PV of chunk s)
            #   q^T block jq  -> before iteration 16jq-2 (scores of jb jq)
            f0 = []
            if JIT:
                for k in range(KC):
                    f0.append((0, v_mm(0, k, st_v)))
                for g in range(1, NJB):
                    for k in range(KC):
                        f0.append((4 * g - 2, qk_mm(4, g, k, st_k0)))
                for s in range(1, NSC):
                    for k in range(KC):
                        f0.append((s, v_mm(s, k, st_v)))
                for jq in range(1, NJB):
                    for k in range(KC):
                        f0.append((16 * jq - 3, qk_mm(0, jq, k, st_q0)))
            # pair 1's q/k production spread over pair 0's later chunks
            make_qk_tile(1)
            make_qk_tile(5)
            st1 = {"q": {}, "k": {}}
            p1 = [qk_mm(1, j, k, st1["q"]) for j in range(NJB)
                  for k in range(KC)]
            p1 += [qk_mm(5, j, k, st1["k"]) for j in range(NJB)
                   for k in range(KC)]
            for i, th in enumerate(p1):
                f0.append((34 + (28 * i) // len(p1), th))
            f0.sort(key=lambda x: x[0])
            emit_B_pair(0, [th for _, th in f0], psA,
                        need=[n for n, _ in f0], budget=4)

            # pairs 1-2, with pair p+1's q/k production dribbled in
            for pair in (1, 2):
                make_qk_tile(pair + 1)
                make_qk_tile(pair + 5)
                st = {"q": {}, "k": {}}
                fl = [qk_mm(pair + 1, j, k, st["q"]) for j in range(NJB)
                      for k in range(KC)]
                fl += [qk_mm(pair + 5, j, k, st["k"]) for j in range(NJB)
                       for k in range(KC)]
                emit_B_pair(pair, fl, psA)

        # ---- pair 3 + projection (xT/wqk/wv freed; wo loads there) ----
        with tc.tile_pool(name="wop", bufs=1) as wop, \
             tc.tile_pool(name="osb", bufs=2) as osb_pool, \
             tc.tile_pool(name="psC", bufs=2, space="PSUM") as psC:
            wo_sb = [wop.tile([P, EMB], X_DT, tag=f"wo{t}", name=f"wo{t}")
                     for t in range(4)]
            for t in range(4):
                nc.sync.dma_start(wo_sb[t][:], wo[t * P:(t + 1) * P, :])

            cstate = {}

            def one_c_mm(s, y, t):
                def go():
                    if t == 0:
                        cstate[(s, y)] = psC.tile([P, 512], F32, tag="psC_t",
                                                  name=f"psc{s}_{y}")
                    ps = cstate[(s, y)]
                    nc.tensor.matmul(
                        ps[:],
                        outT[t][:, s * P:(s + 1) * P],
                        wo_sb[t][:, y * 512:(y + 1) * 512],
                        start=(t == 0), stop=(t == 3))
                    if t == 3:
                        ot = osb_pool.tile([P, 512], BF16, tag="osb",
                                           name=f"osb{s}_{y}")
                        with nc.allow_low_precision(reason="out bf16"):
                            nc.vector.tensor_copy(ot[:], ps[:])
                        nc.sync.dma_start(
                            out[s * P:(s + 1) * P, y * 512:(y + 1) * 512],
                            ot[:])
                return go

            def emit_C_blk(sq0, w):
                return [one_c_mm(s, y, t)
                        for s in range(sq0 // P, (sq0 + w) // P)
                        for y in range(EMB // 512)
                        for t in range(4)]

            blks = None
            if os.environ.get("K_SPLIT", "0") == "1":
                blks = [(0, 512), (512, 512), (1024, 512),
                        (1536, 256), (1792, 256)]
            emit_B_pair(3, [], psC, after_jb=emit_C_blk, dynamic=True,
                        budget=2, flush=True, blocks=blks)

    nc.compile()
    return nc


def get_nc():
    global _CACHED
    if _CACHED is None:
        _CACHED = _build()
    return _CACHED


def make_in_maps(x, W_qkv, b_qkv, W_out, b_out):
    x = np.asarray(x, dtype=np.float32)
    W_qkv = np.asarray(W_qkv, dtype=np.float32)
    b_qkv = np.asarray(b_qkv, dtype=np.float32)
    W_out = np.asarray(W_out, dtype=np.float32)

    import os as _os
    BF = ml_dtypes.bfloat16 if _os.environ.get('K_X', 'bf16') == 'bf16' else np.float32
    in_maps = []
    for core in range(N_CORES):
        b, g = divmod(core, 2)
        c0 = g * 512
        wq = W_qkv[:, c0:c0 + 512] * NORM
        wk = W_qkv[:, EMB + c0:EMB + c0 + 512]
        wv_ = W_qkv[:, 2 * EMB + c0:2 * EMB + c0 + 512]
        bq = b_qkv[c0:c0 + 512] * NORM
        bk = b_qkv[EMB + c0:EMB + c0 + 512]
        bv_ = b_qkv[2 * EMB + c0:2 * EMB + c0 + 512]
        in_maps.append({
            "xT": np.ascontiguousarray(x[b].T).astype(BF),
            "wqk": np.ascontiguousarray(
                np.concatenate([wq, wk], axis=1).reshape(8, P, 8, P)
                .transpose(2, 1, 0, 3).reshape(EMB, QK_COLS)).astype(BF),
            "wv": np.ascontiguousarray(wv_).astype(BF),
            "wo": np.ascontiguousarray(W_out[c0:c0 + 512, :]).astype(BF),
            "bqk": np.ascontiguousarray(
                np.concatenate([bq, bk]).reshape(QK_COLS // P, P).T),
            "bv": bv_.reshape(1, V_COLS).astype(np.float32),
        })
    return in_maps


def kernel(x, W_qkv, b_qkv, W_out, b_out):
    nc = get_nc()
    b_out = np.asarray(b_out, dtype=np.float32)
    in_maps = make_in_maps(x, W_qkv, b_qkv, W_out, b_out)
    res = bass_utils.run_bass_kernel_spmd(nc, in_maps, core_ids=list(range(N_CORES)))
    outp = np.empty((4, SEQ, EMB), dtype=np.float32)
    for b in range(4):
        outp[b] = (res.results[2 * b]["out"].astype(np.float32)
                   + res.results[2 * b + 1]["out"].astype(np.float32)
                   + b_out)
    return outp


# revision 25
# speedup vs baseline: 1.0266x; 1.0266x over previous
"""Multi-head attention (16 heads, E=1024, seq=2048, batch=4) on 8 NeuronCores.

Sharding: core = 2*b + g  (b = batch 0..3, g = head-group 0..1, 8 heads each).
Each core computes its batch's QKV for its 8 heads, attention, and a partial
output projection (rows of W_out for its heads); host sums the two partials
per batch and adds b_out.

On-chip layout avoids all transposes:
  - host supplies x^T [1024, 2048] per core (bf16)
  - q^T,k^T computed as (W^T x^T)  -> [qk_col, seq]   (lhsT = W chunk)
  - v computed naturally as x @ W_v -> [seq, v_col]   (lhsT = x^T chunk)
  - scores^T[sk, sq] = (k^T chunk)^T.T @ q^T  (lhsT = k^T slice, rhs = q^T);
    head pairs share one PSUM tile ([A sq512 | B sq512]) with the two
    64-contraction matmuls row-packed via tile_position (they execute
    concurrently on different PE row groups), so one Exp covers both heads
  - softmax denominator via an appended ones-column in the PV lhsT
  - PV: out^T[d(+1), sq] = [v | 1]^T @ attn^T, accumulated over sk chunks
  - normalize: fast-approx reciprocal of the denominator ROW (1x512) on DVE,
    broadcast across partitions with a K=1 matmul, multiply on DVE; the
    broadcast+multiply are deferred into the next sq-block's chunk stream
  - proj: y[sq, :] from lhsT = out^T tiles, rhs = W_out rows for this group

All weights/activations on the matmul paths are bf16 (full PE rate, FWL
weight loads, half DMA) with fp32 PSUM accumulation; the denominator
reciprocal/broadcast path stays fp32/f32r.

Scheduling: phase B (attention) is paced by the ACT Exp stream, so all
other matmul work (remaining V and q/k for pair 0 via need-by-markers,
next pair's q/k, final projection) is dribbled into the chunk streams to
fill PE slack. The prologue is minimal: q(jb0) + k(jb0) + v(chunk0) right
behind the input DMA, then attention starts and the rest streams in.
"""

import sys

sys.path.insert(0, "/opt/trn_rl_repo")

import ml_dtypes
import numpy as np

import concourse.bacc as bacc
import concourse.mybir as mybir
import concourse.tile as tile
from concourse import bass_utils

P = 128
SEQ = 2048
EMB = 1024
N_HEADS_CORE = 8
D_HEAD = 64
QK_COLS = 1024          # q(512) + k(512) for this core's heads
V_COLS = 512
VA = D_HEAD + 1         # v columns per head incl. ones column
N_CORES = 8
NORM = 0.125            # 1/sqrt(64), folded into W_q/b_q on host

F32 = mybir.dt.float32
F32R = mybir.dt.float32r
BF16 = mybir.dt.bfloat16
AF = mybir.ActivationFunctionType
import os
X_DT = {"bf16": BF16, "f32r": F32R}[os.environ.get("K_X", "bf16")]
QK_DT = {"bf16": BF16, "f32r": F32R}[os.environ.get("K_QK", "bf16")]
AT_DT = {"bf16": BF16, "f32r": F32R}[os.environ.get("K_AT", "bf16")]
V_DT = {"bf16": BF16, "f32r": F32R}[os.environ.get("K_V", "bf16")]
OT_DT = {"bf16": BF16, "f32r": F32R}[os.environ.get("K_OT", "bf16")]
HOST_DT_MAP = {"bf16": None, "f32r": None}

KC = EMB // P          # 8 contraction chunks
NSC = SEQ // P         # 16 seq chunks of 128
NJB = SEQ // 512       # 4 sq blocks of 512

_CACHED = None


def _build():
    nc = bacc.Bacc("TRN2", target_bir_lowering=False, debug=False,
                   enable_asserts=True, num_devices=N_CORES)

    xT = nc.dram_tensor("xT", [EMB, SEQ], X_DT, kind="ExternalInput").ap()
    wqk = nc.dram_tensor("wqk", [EMB, QK_COLS], X_DT, kind="ExternalInput").ap()
    wv = nc.dram_tensor("wv", [EMB, V_COLS], X_DT, kind="ExternalInput").ap()
    wo = nc.dram_tensor("wo", [V_COLS, EMB], X_DT, kind="ExternalInput").ap()
    bqk = nc.dram_tensor("bqk", [P, QK_COLS // P], F32, kind="ExternalInput").ap()
    bv = nc.dram_tensor("bv", [1, V_COLS], F32, kind="ExternalInput").ap()
    out = nc.dram_tensor("out", [SEQ, EMB], BF16, kind="ExternalOutput").ap()

    with tile.TileContext(nc) as tc:
      with tc.tile_pool(name="persist", bufs=1) as persist, \
           tc.tile_pool(name="qkT", bufs=2) as qkT_pool, \
           tc.tile_pool(name="oTp", bufs=1) as oT_pool, \
           tc.tile_pool(name="attn", bufs=3) as attn_pool, \
           tc.tile_pool(name="nrm", bufs=2) as nrm_pool, \
           tc.tile_pool(name="ps_s", bufs=2, space="PSUM") as ps_s_pool, \
           tc.tile_pool(name="ps_o0", bufs=1, space="PSUM") as ps_o0_pool, \
           tc.tile_pool(name="ps_o1", bufs=1, space="PSUM") as ps_o1_pool:
        ps_o_pools = [ps_o0_pool, ps_o1_pool]
        vsb = [persist.tile([P, N_HEADS_CORE * VA], V_DT, tag=f"v{s}", name=f"v{s}")
               for s in range(NSC)]
        bqk_sb = persist.tile([P, QK_COLS // P], F32, tag="bqk")
        bv_sb = persist.tile([P, V_COLS], F32, tag="bv")
        nc.sync.dma_start(bqk_sb[:], bqk)
        nc.sync.dma_start(bv_sb[:], bv[0:1, :].broadcast_to([P, V_COLS]))
        ones_sb = persist.tile([P, D_HEAD], F32R, tag="ones")
        nc.vector.tensor_scalar(ones_sb[:], bv_sb[:, 0:D_HEAD], 0.0, 1.0,
                                mybir.AluOpType.mult, mybir.AluOpType.add)
        ones_f32 = persist.tile([P, 512], F32, tag="ones_f32")
        nc.vector.tensor_scalar(ones_f32[:], bv_sb[:, :], 0.0, 1.0,
                                mybir.AluOpType.mult, mybir.AluOpType.add)
        lndum = persist.tile([1, 1], F32, tag="lndum")
        nc.scalar.activation(lndum[:], ones_f32[0:1, 0:1], AF.Ln)

        qT = {}
        kT = {}
        outT = [oT_pool.tile([P, SEQ], OT_DT, tag=f"oT{t}", name=f"oT{t}")
                for t in range(4)]

        pending = [None]

        def emit_B_pair(t, fillers, scratch_pool, need=None, after_jb=None,
                        dynamic=False, budget=3, flush=False, blocks=None):
            """Head pair (2t, 2t+1): rows 0-63 / 64-127 of qT[t]/kT[t].
            Per chunk one ps_s [128,1024] = [A sq512 | B sq512]; scores
            row-packed, one exp for both heads, PV splits to per-head
            accumulators. `fillers` are thunks sprinkled into the chunk
            stream to fill PE slack under the ACT-paced exp pipeline.
            `need[i]` (optional) = iteration index before whose scores/PV
            filler i must have been emitted (JIT production for pair 0)."""
            kTh = kT[t]
            qTh = qT[t]
            it = 0
            fi = 0
            nfill = len(fillers)
            if blocks is None:
                blocks = [(j * 512, 512) for j in range(NJB)]
            for j, (sq0, w) in enumerate(blocks):
                ps_os = [ps_o_pools[hh].tile([VA, w], F32, tag=f"ps_o{hh}",
                                             name=f"ps_o{t}_{j}_{hh}")
                         for hh in range(2)]

                def scores(c):
                    ps_s = ps_s_pool.tile([P, 2 * w], F32, tag="ps_s",
                                          name=f"ps_s{t}_{j}_{c}")
                    for hh in range(2):
                        pr = hh * D_HEAD
                        nc.tensor.matmul(
                            ps_s[:, hh * w:(hh + 1) * w],
                            kTh[pr:pr + D_HEAD, c * P:(c + 1) * P],
                            qTh[pr:pr + D_HEAD, sq0:sq0 + w],
                            start=True, stop=True, tile_position=(pr, 0))
                    return ps_s

                ps_s = scores(0)
                for c in range(NSC):
                    at = attn_pool.tile([P, 2 * w], AT_DT, tag="attnT",
                                        name=f"at{t}_{j}_{c}")
                    nc.scalar.activation(at[:], ps_s[:], AF.Exp)
                    if need is not None:
                        while fi < nfill and need[fi] <= it:
                            fillers[fi]()
                            fi += 1
                    if c + 1 < NSC:
                        ps_s = scores(c + 1)
                    va3 = vsb[c][:].rearrange("p (h c) -> p h c", c=VA)
                    for hh in range(2):
                        nc.tensor.matmul(
                            ps_os[hh][:],
                            va3[:, 2 * t + hh, :],
                            at[:, hh * w:(hh + 1) * w],
                            start=(c == 0), stop=(c == NSC - 1))
                    it += 1
                    if c == 10 and pending[0] is not None:
                        fin = pending[0]
                        pending[0] = None
                        fin()
                    if dynamic or need is not None:
                        b = budget
                        while fi < len(fillers) and b > 0:
                            fillers[fi]()
                            fi += 1
                            b -= 1
                    else:
                        while nfill and fi < (nfill * it) // 64 and fi < nfill:
                            fillers[fi]()
                            fi += 1

                # stage 1 (DVE): evacuate ps_o FIRST (frees the PSUM banks
                # for the next jb's PV), then ONE reciprocal over both
                # heads' denominator rows (partition-stacked) + f32r cast
                outUs = []
                for hh in range(2):
                    outU = nrm_pool.tile([VA, w], F32, tag=f"outU{hh}",
                                         name=f"outU{t}_{j}_{hh}", bufs=2)
                    nc.vector.tensor_copy(outU[:], ps_os[hh][:])
                    outUs.append(outU)
                if flush and j == len(blocks) - 1:
                    # final block: ACT is idle after the last exp, so the
                    # reciprocal runs there as exp(-ln(d)) — much shorter
                    # serial chain than the DVE reciprocal
                    rc_rows = []
                    for hh in range(2):
                        rln = nrm_pool.tile([VA, w], F32, tag=f"rln{hh}",
                                            name=f"rln{t}_{j}_{hh}", bufs=2)
                        nc.scalar.activation(rln[D_HEAD:VA, :],
                                             outUs[hh][D_HEAD:VA, :], AF.Ln)
                        rca = nrm_pool.tile([VA, w], F32R, tag=f"rca{hh}",
                                            name=f"rca{t}_{j}_{hh}", bufs=2)
                        nc.scalar.activation(rca[D_HEAD:VA, :],
                                             rln[D_HEAD:VA, :], AF.Exp,
                                             scale=-1.0)
                        rc_rows.append((rca, D_HEAD))
                else:
                    # partition-stacked reciprocal: both heads' denominator
                    # rows in one [33,w] tile (rows 0/32), ONE DVE
                    # reciprocal, deferred off the critical path
                    rden = nrm_pool.tile([33, w], F32, tag="rden",
                                         name=f"rden{t}_{j}", bufs=2)
                    nc.vector.tensor_scalar(rden[:], bv_sb[0:33, 0:w],
                                            0.0, 1.0, mybir.AluOpType.mult,
                                            mybir.AluOpType.add)
                    for hh in range(2):
                        nc.vector.tensor_copy(rden[32 * hh:32 * hh + 1, :],
                                              outUs[hh][D_HEAD:VA, :])
                    rrec = nrm_pool.tile([33, w], F32, tag="rrec",
                                         name=f"rrec{t}_{j}", bufs=2)
                    nc.vector.reciprocal(rrec[:], rden[:])
                    rcast = nrm_pool.tile([33, w], F32R, tag="rcast",
                                          name=f"rcast{t}_{j}", bufs=2)
                    with nc.allow_low_precision(reason="denom cast f32r"):
                        nc.vector.tensor_copy(rcast[:], rrec[:])
                    rc_rows = [(rcast, 0), (rcast, 32)]

                # stage 2 (PE bcast + DVE mul): deferred into the NEXT
                # chunk stream (possibly the next pair's) so the PE never
                # waits on the reciprocal chain
                def make_fin(tt, jj, sq00, ww, oUs, rc, ajb):
                    def fin():
                        psb = ps_s_pool.tile([P, 2 * ww], F32, tag="ps_s",
                                             name=f"psb{tt}_{jj}")
                        for hh in range(2):
                            tile_, row = rc[hh]
                            nc.tensor.matmul(psb[0:D_HEAD,
                                                 hh * ww:(hh + 1) * ww],
                                             ones_sb[row:row + 1, :],
                                             tile_[row:row + 1, :],
                                             start=True, stop=True,
                                             tile_position=(row - row % 32,
                                                            0))
                        for hh in range(2):
                            with nc.allow_low_precision(reason="outT bf16"):
                                nc.vector.tensor_mul(
                                    outT[tt][hh * D_HEAD:(hh + 1) * D_HEAD,
                                             sq00:sq00 + ww],
                                    oUs[hh][0:D_HEAD, :],
                                    psb[0:D_HEAD, hh * ww:(hh + 1) * ww])
                        if ajb is not None:
                            fillers.extend(ajb(sq00, ww))
                    return fin

                pending[0] = make_fin(t, j, sq0, w, outUs, rc_rows, after_jb)
            if flush and pending[0] is not None:
                fin = pending[0]
                pending[0] = None
                fin()
            while fi < len(fillers):
                fillers[fi]()
                fi += 1

        # ---- phase A scaffolding (xT, wqk, wv all loaded upfront) ----
        with tc.tile_pool(name="xTp", bufs=1) as xTp, \
             tc.tile_pool(name="wqkp", bufs=1) as wqkp, \
             tc.tile_pool(name="wvp", bufs=1) as wvp, \
             tc.tile_pool(name="psA", bufs=2, space="PSUM") as psA:
            xT_all = xTp.tile([P, KC * SEQ], X_DT, tag="xT", name="xT_all")
            wqkT_sb = [wqkp.tile([P, KC * P], X_DT, tag=f"wqkT{t}",
                                 name=f"wqkT{t}") for t in range(8)]
            wv_all = wvp.tile([P, KC * V_COLS], X_DT, tag="wv", name="wv_all")
            # wqk arrives host-pretiled as [t p, k c]; DMA the two col
            # tiles the first exp needs before everything else, then x
            # (the 4MB long pole), then wv, then the remaining tiles
            for t in (0, 4):
                nc.sync.dma_start(wqkT_sb[t][:], wqk[t * P:(t + 1) * P, :])
            nc.scalar.dma_start(
                xT_all[:].rearrange("p (k c) -> p k c", k=KC),
                xT[:].rearrange("(k p) c -> p k c", p=P))
            nc.scalar.dma_start(
                wv_all[:].rearrange("p (k c) -> p k c", k=KC),
                wv[:].rearrange("(k p) c -> p k c", p=P))
            for t in (1, 5, 2, 6, 3, 7):
                nc.scalar.dma_start(wqkT_sb[t][:], wqk[t * P:(t + 1) * P, :])
            xT_sb = [xT_all[:, k * SEQ:(k + 1) * SEQ] for k in range(KC)]
            wv_sb = [wv_all[:, k * V_COLS:(k + 1) * V_COLS]
                     for k in range(KC)]

            def make_qk_tile(t):
                if t < 4:
                    qT[t] = qkT_pool.tile([P, SEQ], QK_DT, tag="qTa",
                                          name=f"qT{t}")
                else:
                    kT[t - 4] = qkT_pool.tile([P, SEQ], QK_DT, tag="kTa",
                                              name=f"kT{t-4}")

            def qk_mm(t, j, k, state):
                """One matmul of column tile t (q if t<4 else k), sq/sk
                group j, contraction chunk k; bias-add evacuation at k=7."""
                dst = qT[t] if t < 4 else kT[t - 4]

                def go():
                    if k == 0:
                        state[j] = psA.tile([P, 512], F32, tag="psA_t",
                                            name=f"psqk{t}_{j}")
                    ps = state[j]
                    nc.tensor.matmul(
                        ps[:], wqkT_sb[t][:, k * P:(k + 1) * P],
                        xT_sb[k][:, j * 512:(j + 1) * 512],
                        start=(k == 0), stop=(k == KC - 1))
                    if k == KC - 1:
                        with nc.allow_low_precision(reason="qk bf16"):
                            nc.vector.tensor_scalar_add(
                                dst[:, j * 512:(j + 1) * 512], ps[:],
                                bqk_sb[:, t:t + 1])
                return go

            def v_mm(s, k, state):
                def go():
                    if k == 0:
                        state[s] = psA.tile([P, V_COLS], F32, tag="psA_t",
                                            name=f"psv{s}")
                    ps = state[s]
                    nc.tensor.matmul(
                        ps[:], xT_sb[k][:, s * P:(s + 1) * P], wv_sb[k][:],
                        start=(k == 0), stop=(k == KC - 1))
                    if k == KC - 1:
                        v3 = vsb[s][:].rearrange("p (h c) -> p h c", c=VA)
                        ps3 = ps[:].rearrange("p (h c) -> p h c", c=D_HEAD)
                        bv3 = bv_sb[:].rearrange("p (h c) -> p h c", c=D_HEAD)
                        with nc.allow_low_precision(reason="v bf16"):
                            nc.vector.tensor_add(v3[:, :, 0:D_HEAD], ps3, bv3)
                            nc.vector.tensor_scalar(
                                v3[:, :, D_HEAD], bv_sb[:, 0:N_HEADS_CORE],
                                0.0, 1.0, mybir.AluOpType.mult,
                                mybir.AluOpType.add)
                return go

            # minimal prologue: q jb0, k group0, v chunk0 — then attention
            # starts and everything else dribbles into the chunk stream
            JIT = os.environ.get("K_JIT", "1") == "1"
            make_qk_tile(0)
            make_qk_tile(4)
            st_q0, st_k0, st_v = {}, {}, {}
            if JIT:
                for k in range(KC):
                    qk_mm(0, 0, k, st_q0)()
                    qk_mm(4, 0, k, st_k0)()
            else:
                for j in range(NJB):
                    for k in range(KC):
                        qk_mm(0, j, k, st_q0)()
                for g in range(NJB):
                    for k in range(KC):
                        qk_mm(4, g, k, st_k0)()
                for s in range(NSC):
                    for k in range(KC):
                        v_mm(s, k, st_v)()

            # pair 0 fillers with need-by markers (JIT production):
            #   k^T group g   -> before iteration 4g-1 (scores of chunk 4g)
            #   v chunk s     -> before iteration s    (PV of chunk s)
            #   q^T block jq  -> before iteration 16jq-2 (scores of jb jq)
            f0 = []
            if JIT:
                for k in range(KC):
                    f0.append((0, v_mm(0, k, st_v)))
                for g in range(1, NJB):
                    for k in range(KC):
                        f0.append((4 * g - 2, qk_mm(4, g, k, st_k0)))
                for s in range(1, NSC):
                    for k in range(KC):
                        f0.append((s, v_mm(s, k, st_v)))
                for jq in range(1, NJB):
                    for k in range(KC):
                        f0.append((16 * jq - 3, qk_mm(0, jq, k, st_q0)))
            # pair 1's q/k production spread over pair 0's later chunks
            make_qk_tile(1)
            make_qk_tile(5)
            st1 = {"q": {}, "k": {}}
            p1 = [qk_mm(1, j, k, st1["q"]) for j in range(NJB)
                  for k in range(KC)]
            p1 += [qk_mm(5, j, k, st1["k"]) for j in range(NJB)
                   for k in range(KC)]
            for i, th in enumerate(p1):
                f0.append((34 + (28 * i) // len(p1), th))
            f0.sort(key=lambda x: x[0])
            emit_B_pair(0, [th for _, th in f0], psA,
                        need=[n for n, _ in f0], budget=3)

            # pairs 1-2, with pair p+1's q/k production dribbled in
            for pair in (1, 2):
                make_qk_tile(pair + 1)
                make_qk_tile(pair + 5)
                st = {"q": {}, "k": {}}
                fl = [qk_mm(pair + 1, j, k, st["q"]) for j in range(NJB)
                      for k in range(KC)]
                fl += [qk_mm(pair + 5, j, k, st["k"]) for j in range(NJB)
                       for k in range(KC)]
                emit_B_pair(pair, fl, psA)

        # ---- pair 3 + projection (xT/wqk/wv freed; wo loads there) ----
        with tc.tile_pool(name="wop", bufs=1) as wop, \
             tc.tile_pool(name="osb", bufs=2) as osb_pool, \
             tc.tile_pool(name="psC", bufs=2, space="PSUM") as psC:
            wo_sb = [wop.tile([P, EMB], X_DT, tag=f"wo{t}", name=f"wo{t}")
                     for t in range(4)]
            for t in range(4):
                nc.sync.dma_start(wo_sb[t][:], wo[t * P:(t + 1) * P, :])

            cstate = {}

            def one_c_mm(s, y, t):
                def go():
                    if t == 0:
                        cstate[(s, y)] = psC.tile([P, 512], F32, tag="psC_t",
                                                  name=f"psc{s}_{y}")
                    ps = cstate[(s, y)]
                    nc.tensor.matmul(
                        ps[:],
                        outT[t][:, s * P:(s + 1) * P],
                        wo_sb[t][:, y * 512:(y + 1) * 512],
                        start=(t == 0), stop=(t == 3))
                    if t == 3:
                        ot = osb_pool.tile([P, 512], BF16, tag="osb",
                                           name=f"osb{s}_{y}")
                        with nc.allow_low_precision(reason="out bf16"):
                            nc.vector.tensor_copy(ot[:], ps[:])
                        nc.sync.dma_start(
                            out[s * P:(s + 1) * P, y * 512:(y + 1) * 512],
                            ot[:])
                return go

            def emit_C_blk(sq0, w):
                return [one_c_mm(s, y, t)
                        for s in range(sq0 // P, (sq0 + w) // P)
                        for y in range(EMB // 512)
                        for t in range(4)]

            blks = None
            if os.environ.get("K_SPLIT", "0") == "1":
                blks = [(0, 512), (512, 512), (1024, 512),
                        (1536, 256), (1792, 256)]
            emit_B_pair(3, [], psC, after_jb=emit_C_blk, dynamic=True,
                        budget=2, flush=True, blocks=blks)

    nc.compile()
    return nc


def get_nc():
    global _CACHED
    if _CACHED is None:
        _CACHED = _build()
    return _CACHED


def make_in_maps(x, W_qkv, b_qkv, W_out, b_out):
    x = np.asarray(x, dtype=np.float32)
    W_qkv = np.asarray(W_qkv, dtype=np.float32)
    b_qkv = np.asarray(b_qkv, dtype=np.float32)
    W_out = np.asarray(W_out, dtype=np.float32)

    import os as _os
    BF = ml_dtypes.bfloat16 if _os.environ.get('K_X', 'bf16') == 'bf16' else np.float32
    in_maps = []
    for core in range(N_CORES):
        b, g = divmod(core, 2)
        c0 = g * 512
        wq = W_qkv[:, c0:c0 + 512] * NORM
        wk = W_qkv[:, EMB + c0:EMB + c0 + 512]
        wv_ = W_qkv[:, 2 * EMB + c0:2 * EMB + c0 + 512]
        bq = b_qkv[c0:c0 + 512] * NORM
        bk = b_qkv[EMB + c0:EMB + c0 + 512]
        bv_ = b_qkv[2 * EMB + c0:2 * EMB + c0 + 512]
        in_maps.append({
            "xT": np.ascontiguousarray(x[b].T).astype(BF),
            "wqk": np.ascontiguousarray(
                np.concatenate([wq, wk], axis=1).reshape(8, P, 8, P)
                .transpose(2, 1, 0, 3).reshape(EMB, QK_COLS)).astype(BF),
            "wv": np.ascontiguousarray(wv_).astype(BF),
            "wo": np.ascontiguousarray(W_out[c0:c0 + 512, :]).astype(BF),
            "bqk": np.ascontiguousarray(
                np.concatenate([bq, bk]).reshape(QK_COLS // P, P).T),
            "bv": bv_.reshape(1, V_COLS).astype(np.float32),
        })
    return in_maps


def kernel(x, W_qkv, b_qkv, W_out, b_out):
    nc = get_nc()
    b_out = np.asarray(b_out, dtype=np.float32)
    in_maps = make_in_maps(x, W_qkv, b_qkv, W_out, b_out)
    res = bass_utils.run_bass_kernel_spmd(nc, in_maps, core_ids=list(range(N_CORES)))
    outp = np.empty((4, SEQ, EMB), dtype=np.float32)
    for b in range(4):
        outp[b] = (res.results[2 * b]["out"].astype(np.float32)
                   + res.results[2 * b + 1]["out"].astype(np.float32)
                   + b_out)
    return outp


# revision 27
# speedup vs baseline: 1.0269x; 1.0003x over previous
"""Multi-head attention (16 heads, E=1024, seq=2048, batch=4) on 8 NeuronCores.

Sharding: core = 2*b + g  (b = batch 0..3, g = head-group 0..1, 8 heads each).
Each core computes its batch's QKV for its 8 heads, attention, and a partial
output projection (rows of W_out for its heads); host sums the two bf16
partials per batch and adds b_out.

On-chip layout avoids all transposes:
  - host supplies x^T [1024, 2048] per core (bf16) and W_qk pre-tiled per
    128-column tile so each tile is one contiguous DMA
  - q^T,k^T computed as (W^T x^T)  -> [qk_col, seq]   (lhsT = W chunk)
  - v computed naturally as x @ W_v -> [seq, v_col]   (lhsT = x^T chunk)
  - scores^T[sk, sq] = (k^T chunk)^T.T @ q^T  (lhsT = k^T slice, rhs = q^T);
    head pairs share one PSUM tile ([A sq512 | B sq512]) with the two
    64-contraction matmuls row-packed via tile_position (they execute
    concurrently on different PE row groups), so one Exp covers both heads
  - softmax denominator via an appended ones-column in the PV lhsT
  - PV: out^T[d(+1), sq] = [v | 1]^T @ attn^T, accumulated over sk chunks
  - normalize: ONE partition-stacked DVE reciprocal per sq-block (both
    heads' denominator rows at partitions 0/32 of one tile; the custom
    reciprocal_approx ops corrupt data on HW - do not use), broadcast
    across partitions with K=1 matmuls, multiply on DVE; broadcast+multiply
    are deferred into the next chunk stream (across pair boundaries too)
    so the PE never waits on the reciprocal chain. The final block instead
    computes exp(-ln(d)) on the then-idle ACT engine (shorter tail); a
    dummy Ln in the prologue pins the combined ln+exp ACT table.
  - proj: y[sq, :] from lhsT = out^T tiles, rhs = W_out rows for this group

All matmul-path weights/activations are bf16 (full PE rate, FWL weight
loads, half DMA/SBUF) with fp32 PSUM accumulation; output partials are
written bf16 and upconverted on host.

Scheduling: phase B (attention) is paced by the ACT Exp stream (~1.12us
per [128,1024] exp), so all other matmul work is dribbled into the chunk
streams to fill PE slack: remaining V / q^T / k^T production for pair 0
via need-by-markers (JIT, group-atomic to keep the PSUM rotation sound),
the next pair's q/k via a static spread, and the final projection via a
per-chunk budget in pair 3. The prologue is minimal: the two W_qk column
tiles the first scores need DMA first, x^T (the 4MB long pole) and the
rest stream behind, then q(jb0)+k(jb0) right behind the DMA and attention
starts; everything else streams in.
"""

import sys

sys.path.insert(0, "/opt/trn_rl_repo")

import ml_dtypes
import numpy as np

import concourse.bacc as bacc
import concourse.mybir as mybir
import concourse.tile as tile
from concourse import bass_utils

P = 128
SEQ = 2048
EMB = 1024
N_HEADS_CORE = 8
D_HEAD = 64
QK_COLS = 1024          # q(512) + k(512) for this core's heads
V_COLS = 512
VA = D_HEAD + 1         # v columns per head incl. ones column
N_CORES = 8
NORM = 0.125            # 1/sqrt(64), folded into W_q/b_q on host

F32 = mybir.dt.float32
F32R = mybir.dt.float32r
BF16 = mybir.dt.bfloat16
AF = mybir.ActivationFunctionType
import os
X_DT = {"bf16": BF16, "f32r": F32R}[os.environ.get("K_X", "bf16")]
QK_DT = {"bf16": BF16, "f32r": F32R}[os.environ.get("K_QK", "bf16")]
AT_DT = {"bf16": BF16, "f32r": F32R}[os.environ.get("K_AT", "bf16")]
V_DT = {"bf16": BF16, "f32r": F32R}[os.environ.get("K_V", "bf16")]
OT_DT = {"bf16": BF16, "f32r": F32R}[os.environ.get("K_OT", "bf16")]
HOST_DT_MAP = {"bf16": None, "f32r": None}

KC = EMB // P          # 8 contraction chunks
NSC = SEQ // P         # 16 seq chunks of 128
NJB = SEQ // 512       # 4 sq blocks of 512

_CACHED = None


def _build():
    nc = bacc.Bacc("TRN2", target_bir_lowering=False, debug=False,
                   enable_asserts=True, num_devices=N_CORES)

    xT = nc.dram_tensor("xT", [EMB, SEQ], X_DT, kind="ExternalInput").ap()
    wqk = nc.dram_tensor("wqk", [EMB, QK_COLS], X_DT, kind="ExternalInput").ap()
    wv = nc.dram_tensor("wv", [EMB, V_COLS], X_DT, kind="ExternalInput").ap()
    wo = nc.dram_tensor("wo", [V_COLS, EMB], X_DT, kind="ExternalInput").ap()
    bqk = nc.dram_tensor("bqk", [P, QK_COLS // P], F32, kind="ExternalInput").ap()
    bv = nc.dram_tensor("bv", [1, V_COLS], F32, kind="ExternalInput").ap()
    out = nc.dram_tensor("out", [SEQ, EMB], BF16, kind="ExternalOutput").ap()

    with tile.TileContext(nc) as tc:
      with tc.tile_pool(name="persist", bufs=1) as persist, \
           tc.tile_pool(name="qkT", bufs=2) as qkT_pool, \
           tc.tile_pool(name="oTp", bufs=1) as oT_pool, \
           tc.tile_pool(name="attn", bufs=3) as attn_pool, \
           tc.tile_pool(name="nrm", bufs=2) as nrm_pool, \
           tc.tile_pool(name="ps_s", bufs=2, space="PSUM") as ps_s_pool, \
           tc.tile_pool(name="ps_o0", bufs=1, space="PSUM") as ps_o0_pool, \
           tc.tile_pool(name="ps_o1", bufs=1, space="PSUM") as ps_o1_pool:
        ps_o_pools = [ps_o0_pool, ps_o1_pool]
        vsb = [persist.tile([P, N_HEADS_CORE * VA], V_DT, tag=f"v{s}", name=f"v{s}")
               for s in range(NSC)]
        bqk_sb = persist.tile([P, QK_COLS // P], F32, tag="bqk")
        bv_sb = persist.tile([P, V_COLS], F32, tag="bv")
        nc.sync.dma_start(bqk_sb[:], bqk)
        nc.sync.dma_start(bv_sb[:], bv[0:1, :].broadcast_to([P, V_COLS]))
        ones_sb = persist.tile([P, D_HEAD], F32R, tag="ones")
        nc.vector.tensor_scalar(ones_sb[:], bv_sb[:, 0:D_HEAD], 0.0, 1.0,
                                mybir.AluOpType.mult, mybir.AluOpType.add)
        ones_f32 = persist.tile([P, 512], F32, tag="ones_f32")
        nc.vector.tensor_scalar(ones_f32[:], bv_sb[:, :], 0.0, 1.0,
                                mybir.AluOpType.mult, mybir.AluOpType.add)
        lndum = persist.tile([1, 1], F32, tag="lndum")
        nc.scalar.activation(lndum[:], ones_f32[0:1, 0:1], AF.Ln)

        qT = {}
        kT = {}
        outT = [oT_pool.tile([P, SEQ], OT_DT, tag=f"oT{t}", name=f"oT{t}")
                for t in range(4)]

        pending = [None]

        def emit_B_pair(t, fillers, scratch_pool, need=None, after_jb=None,
                        dynamic=False, budget=3, flush=False, blocks=None):
            """Head pair (2t, 2t+1): rows 0-63 / 64-127 of qT[t]/kT[t].
            Per chunk one ps_s [128,1024] = [A sq512 | B sq512]; scores
            row-packed, one exp for both heads, PV splits to per-head
            accumulators. `fillers` are thunks sprinkled into the chunk
            stream to fill PE slack under the ACT-paced exp pipeline.
            `need[i]` (optional) = iteration index before whose scores/PV
            filler i must have been emitted (JIT production for pair 0)."""
            kTh = kT[t]
            qTh = qT[t]
            it = 0
            fi = 0
            nfill = len(fillers)
            if blocks is None:
                blocks = [(j * 512, 512) for j in range(NJB)]
            for j, (sq0, w) in enumerate(blocks):
                ps_os = [ps_o_pools[hh].tile([VA, w], F32, tag=f"ps_o{hh}",
                                             name=f"ps_o{t}_{j}_{hh}")
                         for hh in range(2)]

                def scores(c):
                    ps_s = ps_s_pool.tile([P, 2 * w], F32, tag="ps_s",
                                          name=f"ps_s{t}_{j}_{c}")
                    for hh in range(2):
                        pr = hh * D_HEAD
                        nc.tensor.matmul(
                            ps_s[:, hh * w:(hh + 1) * w],
                            kTh[pr:pr + D_HEAD, c * P:(c + 1) * P],
                            qTh[pr:pr + D_HEAD, sq0:sq0 + w],
                            start=True, stop=True, tile_position=(pr, 0))
                    return ps_s

                ps_s = scores(0)
                for c in range(NSC):
                    at = attn_pool.tile([P, 2 * w], AT_DT, tag="attnT",
                                        name=f"at{t}_{j}_{c}")
                    nc.scalar.activation(at[:], ps_s[:], AF.Exp)
                    if need is not None:
                        nb = 8
                        while fi < nfill and need[fi] <= it and nb > 0:
                            fillers[fi]()
                            fi += 1
                            nb -= 1
                    if c + 1 < NSC:
                        ps_s = scores(c + 1)
                    if need is not None:
                        while fi < nfill and need[fi] <= it:
                            fillers[fi]()
                            fi += 1
                    va3 = vsb[c][:].rearrange("p (h c) -> p h c", c=VA)
                    for hh in range(2):
                        nc.tensor.matmul(
                            ps_os[hh][:],
                            va3[:, 2 * t + hh, :],
                            at[:, hh * w:(hh + 1) * w],
                            start=(c == 0), stop=(c == NSC - 1))
                    it += 1
                    if c == 10 and pending[0] is not None:
                        fin = pending[0]
                        pending[0] = None
                        fin()
                    if dynamic or need is not None:
                        b = budget
                        while fi < len(fillers) and b > 0:
                            fillers[fi]()
                            fi += 1
                            b -= 1
                    else:
                        while nfill and fi < (nfill * it) // 64 and fi < nfill:
                            fillers[fi]()
                            fi += 1

                # stage 1 (DVE): evacuate ps_o FIRST (frees the PSUM banks
                # for the next jb's PV), then ONE reciprocal over both
                # heads' denominator rows (partition-stacked) + f32r cast
                outUs = []
                for hh in range(2):
                    outU = nrm_pool.tile([VA, w], F32, tag=f"outU{hh}",
                                         name=f"outU{t}_{j}_{hh}", bufs=2)
                    nc.vector.tensor_copy(outU[:], ps_os[hh][:])
                    outUs.append(outU)
                if flush and j == len(blocks) - 1:
                    # final block: ACT is idle after the last exp, so the
                    # reciprocal runs there as exp(-ln(d)) — much shorter
                    # serial chain than the DVE reciprocal
                    rc_rows = []
                    for hh in range(2):
                        rln = nrm_pool.tile([VA, w], F32, tag=f"rln{hh}",
                                            name=f"rln{t}_{j}_{hh}", bufs=2)
                        nc.scalar.activation(rln[D_HEAD:VA, :],
                                             outUs[hh][D_HEAD:VA, :], AF.Ln)
                        rca = nrm_pool.tile([VA, w], F32R, tag=f"rca{hh}",
                                            name=f"rca{t}_{j}_{hh}", bufs=2)
                        nc.scalar.activation(rca[D_HEAD:VA, :],
                                             rln[D_HEAD:VA, :], AF.Exp,
                                             scale=-1.0)
                        rc_rows.append((rca, D_HEAD))
                else:
                    # partition-stacked reciprocal: both heads' denominator
                    # rows in one [33,w] tile (rows 0/32), ONE DVE
                    # reciprocal, deferred off the critical path
                    rden = nrm_pool.tile([33, w], F32, tag="rden",
                                         name=f"rden{t}_{j}", bufs=2)
                    nc.vector.tensor_scalar(rden[:], bv_sb[0:33, 0:w],
                                            0.0, 1.0, mybir.AluOpType.mult,
                                            mybir.AluOpType.add)
                    for hh in range(2):
                        nc.vector.tensor_copy(rden[32 * hh:32 * hh + 1, :],
                                              outUs[hh][D_HEAD:VA, :])
                    rrec = nrm_pool.tile([33, w], F32, tag="rrec",
                                         name=f"rrec{t}_{j}", bufs=2)
                    nc.vector.reciprocal(rrec[:], rden[:])
                    rcast = nrm_pool.tile([33, w], F32R, tag="rcast",
                                          name=f"rcast{t}_{j}", bufs=2)
                    with nc.allow_low_precision(reason="denom cast f32r"):
                        nc.vector.tensor_copy(rcast[:], rrec[:])
                    rc_rows = [(rcast, 0), (rcast, 32)]

                # stage 2 (PE bcast + DVE mul): deferred into the NEXT
                # chunk stream (possibly the next pair's) so the PE never
                # waits on the reciprocal chain
                def make_fin(tt, jj, sq00, ww, oUs, rc, ajb):
                    def fin():
                        psb = ps_s_pool.tile([P, 2 * ww], F32, tag="ps_s",
                                             name=f"psb{tt}_{jj}")
                        for hh in range(2):
                            tile_, row = rc[hh]
                            nc.tensor.matmul(psb[0:D_HEAD,
                                                 hh * ww:(hh + 1) * ww],
                                             ones_sb[row:row + 1, :],
                                             tile_[row:row + 1, :],
                                             start=True, stop=True,
                                             tile_position=(row - row % 32,
                                                            0))
                        for hh in range(2):
                            with nc.allow_low_precision(reason="outT bf16"):
                                nc.vector.tensor_mul(
                                    outT[tt][hh * D_HEAD:(hh + 1) * D_HEAD,
                                             sq00:sq00 + ww],
                                    oUs[hh][0:D_HEAD, :],
                                    psb[0:D_HEAD, hh * ww:(hh + 1) * ww])
                        if ajb is not None:
                            fillers.extend(ajb(sq00, ww))
                    return fin

                pending[0] = make_fin(t, j, sq0, w, outUs, rc_rows, after_jb)
            if flush and pending[0] is not None:
                fin = pending[0]
                pending[0] = None
                fin()
            while fi < len(fillers):
                fillers[fi]()
                fi += 1

        # ---- phase A scaffolding (xT, wqk, wv all loaded upfront) ----
        with tc.tile_pool(name="xTp", bufs=1) as xTp, \
             tc.tile_pool(name="wqkp", bufs=1) as wqkp, \
             tc.tile_pool(name="wvp", bufs=1) as wvp, \
             tc.tile_pool(name="psA", bufs=2, space="PSUM") as psA:
            xT_all = xTp.tile([P, KC * SEQ], X_DT, tag="xT", name="xT_all")
            wqkT_sb = [wqkp.tile([P, KC * P], X_DT, tag=f"wqkT{t}",
                                 name=f"wqkT{t}") for t in range(8)]
            wv_all = wvp.tile([P, KC * V_COLS], X_DT, tag="wv", name="wv_all")
            # wqk arrives host-pretiled as [t p, k c]; DMA the two col
            # tiles the first exp needs before everything else, then x
            # (the 4MB long pole), then wv, then the remaining tiles
            for t in (0, 4):
                nc.sync.dma_start(wqkT_sb[t][:], wqk[t * P:(t + 1) * P, :])
            half = (KC // 2) * P
            for h in range(2):
                nc.scalar.dma_start(
                    xT_all[:, h * 4 * SEQ:(h + 1) * 4 * SEQ].rearrange(
                        "p (k c) -> p k c", k=KC // 2),
                    xT[h * half:(h + 1) * half, :].rearrange(
                        "(k p) c -> p k c", p=P))
            nc.scalar.dma_start(
                wv_all[:].rearrange("p (k c) -> p k c", k=KC),
                wv[:].rearrange("(k p) c -> p k c", p=P))
            for t in (1, 5, 2, 6, 3, 7):
                nc.scalar.dma_start(wqkT_sb[t][:], wqk[t * P:(t + 1) * P, :])
            xT_sb = [xT_all[:, k * SEQ:(k + 1) * SEQ] for k in range(KC)]
            wv_sb = [wv_all[:, k * V_COLS:(k + 1) * V_COLS]
                     for k in range(KC)]

            def make_qk_tile(t):
                if t < 4:
                    qT[t] = qkT_pool.tile([P, SEQ], QK_DT, tag="qTa",
                                          name=f"qT{t}")
                else:
                    kT[t - 4] = qkT_pool.tile([P, SEQ], QK_DT, tag="kTa",
                                              name=f"kT{t-4}")

            def qk_mm(t, j, k, state):
                """One matmul of column tile t (q if t<4 else k), sq/sk
                group j, contraction chunk k; bias-add evacuation at k=7."""
                dst = qT[t] if t < 4 else kT[t - 4]

                def go():
                    if k == 0:
                        state[j] = psA.tile([P, 512], F32, tag="psA_t",
                                            name=f"psqk{t}_{j}")
                    ps = state[j]
                    nc.tensor.matmul(
                        ps[:], wqkT_sb[t][:, k * P:(k + 1) * P],
                        xT_sb[k][:, j * 512:(j + 1) * 512],
                        start=(k == 0), stop=(k == KC - 1))
                    if k == KC - 1:
                        with nc.allow_low_precision(reason="qk bf16"):
                            nc.vector.tensor_scalar_add(
                                dst[:, j * 512:(j + 1) * 512], ps[:],
                                bqk_sb[:, t:t + 1])
                return go

            def v_mm(s, k, state):
                def go():
                    if k == 0:
                        state[s] = psA.tile([P, V_COLS], F32, tag="psA_t",
                                            name=f"psv{s}")
                    ps = state[s]
                    nc.tensor.matmul(
                        ps[:], xT_sb[k][:, s * P:(s + 1) * P], wv_sb[k][:],
                        start=(k == 0), stop=(k == KC - 1))
                    if k == KC - 1:
                        v3 = vsb[s][:].rearrange("p (h c) -> p h c", c=VA)
                        ps3 = ps[:].rearrange("p (h c) -> p h c", c=D_HEAD)
                        bv3 = bv_sb[:].rearrange("p (h c) -> p h c", c=D_HEAD)
                        with nc.allow_low_precision(reason="v bf16"):
                            nc.vector.tensor_add(v3[:, :, 0:D_HEAD], ps3, bv3)
                            nc.vector.tensor_scalar(
                                v3[:, :, D_HEAD], bv_sb[:, 0:N_HEADS_CORE],
                                0.0, 1.0, mybir.AluOpType.mult,
                                mybir.AluOpType.add)
                return go

            # minimal prologue: q jb0, k group0, v chunk0 — then attention
            # starts and everything else dribbles into the chunk stream
            JIT = os.environ.get("K_JIT", "1") == "1"
            make_qk_tile(0)
            make_qk_tile(4)
            st_q0, st_k0, st_v = {}, {}, {}
            if JIT:
                for k in range(KC):
                    qk_mm(0, 0, k, st_q0)()
                    qk_mm(4, 0, k, st_k0)()
            else:
                for j in range(NJB):
                    for k in range(KC):
                        qk_mm(0, j, k, st_q0)()
                for g in range(NJB):
                    for k in range(KC):
                        qk_mm(4, g, k, st_k0)()
                for s in range(NSC):
                    for k in range(KC):
                        v_mm(s, k, st_v)()

            # pair 0 fillers with need-by markers (JIT production):
            #   k^T group g   -> before iteration 4g-1 (scores of chunk 4g)
            #   v chunk s     -> before iteration s    (PV of chunk s)
            #   q^T block jq  -> before iteration 16jq-2 (scores of jb jq)
            f0 = []
            if JIT:
                for k in range(KC):
                    f0.append((0, v_mm(0, k, st_v)))
                for g in range(1, NJB):
                    for k in range(KC):
                        f0.append((4 * g - 2, qk_mm(4, g, k, st_k0)))
                for s in range(1, NSC):
                    for k in range(KC):
                        f0.append((s, v_mm(s, k, st_v)))
                for jq in range(1, NJB):
                    for k in range(KC):
                        f0.append((16 * jq - 3, qk_mm(0, jq, k, st_q0)))
            # pair 1's q/k production spread over pair 0's later chunks
            make_qk_tile(1)
            make_qk_tile(5)
            st1 = {"q": {}, "k": {}}
            p1 = [qk_mm(1, j, k, st1["q"]) for j in range(NJB)
                  for k in range(KC)]
            p1 += [qk_mm(5, j, k, st1["k"]) for j in range(NJB)
                   for k in range(KC)]
            for i, th in enumerate(p1):
                f0.append((34 + (28 * i) // len(p1), th))
            f0.sort(key=lambda x: x[0])
            emit_B_pair(0, [th for _, th in f0], psA,
                        need=[n for n, _ in f0], budget=3)

            # pairs 1-2, with pair p+1's q/k production dribbled in
            for pair in (1, 2):
                make_qk_tile(pair + 1)
                make_qk_tile(pair + 5)
                st = {"q": {}, "k": {}}
                fl = [qk_mm(pair + 1, j, k, st["q"]) for j in range(NJB)
                      for k in range(KC)]
                fl += [qk_mm(pair + 5, j, k, st["k"]) for j in range(NJB)
                       for k in range(KC)]
                emit_B_pair(pair, fl, psA)

        # ---- pair 3 + projection (xT/wqk/wv freed; wo loads there) ----
        with tc.tile_pool(name="wop", bufs=1) as wop, \
             tc.tile_pool(name="osb", bufs=2) as osb_pool, \
             tc.tile_pool(name="psC", bufs=2, space="PSUM") as psC:
            wo_sb = [wop.tile([P, EMB], X_DT, tag=f"wo{t}", name=f"wo{t}")
                     for t in range(4)]
            for t in range(4):
                nc.sync.dma_start(wo_sb[t][:], wo[t * P:(t + 1) * P, :])

            cstate = {}

            def one_c_mm(s, y, t):
                def go():
                    if t == 0:
                        cstate[(s, y)] = psC.tile([P, 512], F32, tag="psC_t",
                                                  name=f"psc{s}_{y}")
                    ps = cstate[(s, y)]
                    nc.tensor.matmul(
                        ps[:],
                        outT[t][:, s * P:(s + 1) * P],
                        wo_sb[t][:, y * 512:(y + 1) * 512],
                        start=(t == 0), stop=(t == 3))
                    if t == 3:
                        ot = osb_pool.tile([P, 512], BF16, tag="osb",
                                           name=f"osb{s}_{y}")
                        with nc.allow_low_precision(reason="out bf16"):
                            nc.vector.tensor_copy(ot[:], ps[:])
                        nc.sync.dma_start(
                            out[s * P:(s + 1) * P, y * 512:(y + 1) * 512],
                            ot[:])
                return go

            def emit_C_blk(sq0, w):
                return [one_c_mm(s, y, t)
                        for s in range(sq0 // P, (sq0 + w) // P)
                        for y in range(EMB // 512)
                        for t in range(4)]

            blks = None
            if os.environ.get("K_SPLIT", "0") == "1":
                blks = [(0, 512), (512, 512), (1024, 512),
                        (1536, 256), (1792, 256)]
            emit_B_pair(3, [], psC, after_jb=emit_C_blk, dynamic=True,
                        budget=2, flush=True, blocks=blks)

    nc.compile()
    return nc


def get_nc():
    global _CACHED
    if _CACHED is None:
        _CACHED = _build()
    return _CACHED


def make_in_maps(x, W_qkv, b_qkv, W_out, b_out):
    x = np.asarray(x, dtype=np.float32)
    W_qkv = np.asarray(W_qkv, dtype=np.float32)
    b_qkv = np.asarray(b_qkv, dtype=np.float32)
    W_out = np.asarray(W_out, dtype=np.float32)

    import os as _os
    BF = ml_dtypes.bfloat16 if _os.environ.get('K_X', 'bf16') == 'bf16' else np.float32
    in_maps = []
    for core in range(N_CORES):
        b, g = divmod(core, 2)
        c0 = g * 512
        wq = W_qkv[:, c0:c0 + 512] * NORM
        wk = W_qkv[:, EMB + c0:EMB + c0 + 512]
        wv_ = W_qkv[:, 2 * EMB + c0:2 * EMB + c0 + 512]
        bq = b_qkv[c0:c0 + 512] * NORM
        bk = b_qkv[EMB + c0:EMB + c0 + 512]
        bv_ = b_qkv[2 * EMB + c0:2 * EMB + c0 + 512]
        in_maps.append({
            "xT": np.ascontiguousarray(x[b].T).astype(BF),
            "wqk": np.ascontiguousarray(
                np.concatenate([wq, wk], axis=1).reshape(8, P, 8, P)
                .transpose(2, 1, 0, 3).reshape(EMB, QK_COLS)).astype(BF),
            "wv": np.ascontiguousarray(wv_).astype(BF),
            "wo": np.ascontiguousarray(W_out[c0:c0 + 512, :]).astype(BF),
            "bqk": np.ascontiguousarray(
                np.concatenate([bq, bk]).reshape(QK_COLS // P, P).T),
            "bv": bv_.reshape(1, V_COLS).astype(np.float32),
        })
    return in_maps


def kernel(x, W_qkv, b_qkv, W_out, b_out):
    nc = get_nc()
    b_out = np.asarray(b_out, dtype=np.float32)
    in_maps = make_in_maps(x, W_qkv, b_qkv, W_out, b_out)
    res = bass_utils.run_bass_kernel_spmd(nc, in_maps, core_ids=list(range(N_CORES)))
    outp = np.empty((4, SEQ, EMB), dtype=np.float32)
    for b in range(4):
        outp[b] = (res.results[2 * b]["out"].astype(np.float32)
                   + res.results[2 * b + 1]["out"].astype(np.float32)
                   + b_out)
    return outp


# revision 28
# speedup vs baseline: 1.0363x; 1.0091x over previous
"""Multi-head attention (16 heads, E=1024, seq=2048, batch=4) on 8 NeuronCores.

Sharding: core = 2*b + g  (b = batch 0..3, g = head-group 0..1, 8 heads each).
Each core computes its batch's QKV for its 8 heads, attention, and a partial
output projection (rows of W_out for its heads); host sums the two bf16
partials per batch and adds b_out.

On-chip layout avoids all transposes:
  - host supplies x^T [1024, 2048] per core (bf16) and W_qk pre-tiled per
    128-column tile so each tile is one contiguous DMA
  - q^T,k^T computed as (W^T x^T)  -> [qk_col, seq]   (lhsT = W chunk)
  - v computed naturally as x @ W_v -> [seq, v_col]   (lhsT = x^T chunk)
  - scores^T[sk, sq] = (k^T chunk)^T.T @ q^T  (lhsT = k^T slice, rhs = q^T);
    head pairs share one PSUM tile ([A sq512 | B sq512]) with the two
    64-contraction matmuls row-packed via tile_position (they execute
    concurrently on different PE row groups), so one Exp covers both heads
  - softmax denominator via an appended ones-column in the PV lhsT
  - PV: out^T[d(+1), sq] = [v | 1]^T @ attn^T, accumulated over sk chunks
  - normalize: ONE partition-stacked DVE reciprocal per sq-block (both
    heads' denominator rows at partitions 0/32 of one tile; the custom
    reciprocal_approx ops corrupt data on HW - do not use), broadcast
    across partitions with K=1 matmuls, multiply on DVE; broadcast+multiply
    are deferred into the next chunk stream (across pair boundaries too)
    so the PE never waits on the reciprocal chain. The final block instead
    computes exp(-ln(d)) on the then-idle ACT engine (shorter tail); a
    dummy Ln in the prologue pins the combined ln+exp ACT table.
  - proj: y[sq, :] from lhsT = out^T tiles, rhs = W_out rows for this group

All matmul-path weights/activations are bf16 (full PE rate, FWL weight
loads, half DMA/SBUF) with fp32 PSUM accumulation; output partials are
written bf16 and upconverted on host.

Scheduling: phase B (attention) is paced by the ACT Exp stream (~1.12us
per [128,1024] exp), so all other matmul work is dribbled into the chunk
streams to fill PE slack: remaining V / q^T / k^T production for pair 0
via need-by-markers (JIT, group-atomic to keep the PSUM rotation sound),
the next pair's q/k via a static spread, and the final projection via a
per-chunk budget in pair 3. The prologue is minimal: the two W_qk column
tiles the first scores need DMA first, x^T (the 4MB long pole) and the
rest stream behind, then q(jb0)+k(jb0) right behind the DMA and attention
starts; everything else streams in.
"""

import sys

sys.path.insert(0, "/opt/trn_rl_repo")

import ml_dtypes
import numpy as np

import concourse.bacc as bacc
import concourse.mybir as mybir
import concourse.tile as tile
from concourse import bass_utils

P = 128
SEQ = 2048
EMB = 1024
N_HEADS_CORE = 8
D_HEAD = 64
QK_COLS = 1024          # q(512) + k(512) for this core's heads
V_COLS = 512
VA = D_HEAD + 1         # v columns per head incl. ones column
N_CORES = 8
NORM = 0.125            # 1/sqrt(64), folded into W_q/b_q on host

F32 = mybir.dt.float32
F32R = mybir.dt.float32r
BF16 = mybir.dt.bfloat16
AF = mybir.ActivationFunctionType
import os
X_DT = {"bf16": BF16, "f32r": F32R}[os.environ.get("K_X", "bf16")]
QK_DT = {"bf16": BF16, "f32r": F32R}[os.environ.get("K_QK", "bf16")]
AT_DT = {"bf16": BF16, "f32r": F32R}[os.environ.get("K_AT", "bf16")]
V_DT = {"bf16": BF16, "f32r": F32R}[os.environ.get("K_V", "bf16")]
OT_DT = {"bf16": BF16, "f32r": F32R}[os.environ.get("K_OT", "bf16")]
HOST_DT_MAP = {"bf16": None, "f32r": None}

KC = EMB // P          # 8 contraction chunks
NSC = SEQ // P         # 16 seq chunks of 128
NJB = SEQ // 512       # 4 sq blocks of 512

_CACHED = None


def _build():
    nc = bacc.Bacc("TRN2", target_bir_lowering=False, debug=False,
                   enable_asserts=True, num_devices=N_CORES)

    xT = nc.dram_tensor("xT", [EMB, SEQ], X_DT, kind="ExternalInput").ap()
    wqk = nc.dram_tensor("wqk", [EMB, QK_COLS], X_DT, kind="ExternalInput").ap()
    wv = nc.dram_tensor("wv", [EMB, V_COLS], X_DT, kind="ExternalInput").ap()
    wo = nc.dram_tensor("wo", [V_COLS, EMB], X_DT, kind="ExternalInput").ap()
    bqk = nc.dram_tensor("bqk", [P, QK_COLS // P], F32, kind="ExternalInput").ap()
    bv = nc.dram_tensor("bv", [1, V_COLS], F32, kind="ExternalInput").ap()
    out = nc.dram_tensor("out", [SEQ, EMB], BF16, kind="ExternalOutput").ap()

    with tile.TileContext(nc) as tc:
      with tc.tile_pool(name="persist", bufs=1) as persist, \
           tc.tile_pool(name="qkT", bufs=2) as qkT_pool, \
           tc.tile_pool(name="oTp", bufs=1) as oT_pool, \
           tc.tile_pool(name="attn", bufs=4) as attn_pool, \
           tc.tile_pool(name="nrm", bufs=2) as nrm_pool, \
           tc.tile_pool(name="ps_s", bufs=2, space="PSUM") as ps_s_pool, \
           tc.tile_pool(name="ps_o0", bufs=1, space="PSUM") as ps_o0_pool, \
           tc.tile_pool(name="ps_o1", bufs=1, space="PSUM") as ps_o1_pool:
        ps_o_pools = [ps_o0_pool, ps_o1_pool]
        vsb = [persist.tile([P, N_HEADS_CORE * VA], V_DT, tag=f"v{s}", name=f"v{s}")
               for s in range(NSC)]
        bqk_sb = persist.tile([P, QK_COLS // P], F32, tag="bqk")
        bv_sb = persist.tile([P, V_COLS], F32, tag="bv")
        nc.sync.dma_start(bqk_sb[:], bqk)
        nc.sync.dma_start(bv_sb[:], bv[0:1, :].broadcast_to([P, V_COLS]))
        ones_sb = persist.tile([P, D_HEAD], F32R, tag="ones")
        nc.vector.tensor_scalar(ones_sb[:], bv_sb[:, 0:D_HEAD], 0.0, 1.0,
                                mybir.AluOpType.mult, mybir.AluOpType.add)
        ones_f32 = persist.tile([P, 512], F32, tag="ones_f32")
        nc.vector.tensor_scalar(ones_f32[:], bv_sb[:, :], 0.0, 1.0,
                                mybir.AluOpType.mult, mybir.AluOpType.add)
        lndum = persist.tile([1, 1], F32, tag="lndum")
        nc.scalar.activation(lndum[:], ones_f32[0:1, 0:1], AF.Ln)

        qT = {}
        kT = {}
        outT = [oT_pool.tile([P, SEQ], OT_DT, tag=f"oT{t}", name=f"oT{t}")
                for t in range(4)]

        pending = [None]

        def emit_B_pair(t, fillers, scratch_pool, need=None, after_jb=None,
                        dynamic=False, budget=3, flush=False, blocks=None):
            """Head pair (2t, 2t+1): rows 0-63 / 64-127 of qT[t]/kT[t].
            Per chunk one ps_s [128,1024] = [A sq512 | B sq512]; scores
            row-packed, one exp for both heads, PV splits to per-head
            accumulators. `fillers` are thunks sprinkled into the chunk
            stream to fill PE slack under the ACT-paced exp pipeline.
            `need[i]` (optional) = iteration index before whose scores/PV
            filler i must have been emitted (JIT production for pair 0)."""
            kTh = kT[t]
            qTh = qT[t]
            it = 0
            fi = 0
            nfill = len(fillers)
            if blocks is None:
                blocks = [(j * 512, 512) for j in range(NJB)]
            for j, (sq0, w) in enumerate(blocks):
                ps_os = [ps_o_pools[hh].tile([VA, w], F32, tag=f"ps_o{hh}",
                                             name=f"ps_o{t}_{j}_{hh}")
                         for hh in range(2)]

                def scores(c):
                    ps_s = ps_s_pool.tile([P, 2 * w], F32, tag="ps_s",
                                          name=f"ps_s{t}_{j}_{c}")
                    for hh in range(2):
                        pr = hh * D_HEAD
                        nc.tensor.matmul(
                            ps_s[:, hh * w:(hh + 1) * w],
                            kTh[pr:pr + D_HEAD, c * P:(c + 1) * P],
                            qTh[pr:pr + D_HEAD, sq0:sq0 + w],
                            start=True, stop=True, tile_position=(pr, 0))
                    return ps_s

                ps_s = scores(0)
                for c in range(NSC):
                    at = attn_pool.tile([P, 2 * w], AT_DT, tag="attnT",
                                        name=f"at{t}_{j}_{c}")
                    nc.scalar.activation(at[:], ps_s[:], AF.Exp)
                    if need is not None:
                        nb = 8
                        while fi < nfill and need[fi] <= it and nb > 0:
                            fillers[fi]()
                            fi += 1
                            nb -= 1
                    if c + 1 < NSC:
                        ps_s = scores(c + 1)
                    if need is not None:
                        while fi < nfill and need[fi] <= it:
                            fillers[fi]()
                            fi += 1
                    va3 = vsb[c][:].rearrange("p (h c) -> p h c", c=VA)
                    for hh in range(2):
                        nc.tensor.matmul(
                            ps_os[hh][:],
                            va3[:, 2 * t + hh, :],
                            at[:, hh * w:(hh + 1) * w],
                            start=(c == 0), stop=(c == NSC - 1))
                    it += 1
                    if c == 10 and pending[0] is not None:
                        fin = pending[0]
                        pending[0] = None
                        fin()
                    if dynamic or need is not None:
                        b = budget
                        while fi < len(fillers) and b > 0:
                            fillers[fi]()
                            fi += 1
                            b -= 1
                    else:
                        while nfill and fi < (nfill * it) // 64 and fi < nfill:
                            fillers[fi]()
                            fi += 1

                # stage 1 (DVE): evacuate ps_o FIRST (frees the PSUM banks
                # for the next jb's PV), then ONE reciprocal over both
                # heads' denominator rows (partition-stacked) + f32r cast
                outUs = []
                for hh in range(2):
                    outU = nrm_pool.tile([VA, w], F32, tag=f"outU{hh}",
                                         name=f"outU{t}_{j}_{hh}", bufs=2)
                    nc.vector.tensor_copy(outU[:], ps_os[hh][:])
                    outUs.append(outU)
                if flush and j == len(blocks) - 1:
                    # final block: ACT is idle after the last exp, so the
                    # reciprocal runs there as exp(-ln(d)) — much shorter
                    # serial chain than the DVE reciprocal
                    rc_rows = []
                    for hh in range(2):
                        rln = nrm_pool.tile([VA, w], F32, tag=f"rln{hh}",
                                            name=f"rln{t}_{j}_{hh}", bufs=2)
                        nc.scalar.activation(rln[D_HEAD:VA, :],
                                             outUs[hh][D_HEAD:VA, :], AF.Ln)
                        rca = nrm_pool.tile([VA, w], F32R, tag=f"rca{hh}",
                                            name=f"rca{t}_{j}_{hh}", bufs=2)
                        nc.scalar.activation(rca[D_HEAD:VA, :],
                                             rln[D_HEAD:VA, :], AF.Exp,
                                             scale=-1.0)
                        rc_rows.append((rca, D_HEAD))
                else:
                    # partition-stacked reciprocal: both heads' denominator
                    # rows in one [33,w] tile (rows 0/32), ONE DVE
                    # reciprocal, deferred off the critical path
                    rden = nrm_pool.tile([33, w], F32, tag="rden",
                                         name=f"rden{t}_{j}", bufs=2)
                    nc.vector.tensor_scalar(rden[:], bv_sb[0:33, 0:w],
                                            0.0, 1.0, mybir.AluOpType.mult,
                                            mybir.AluOpType.add)
                    for hh in range(2):
                        nc.vector.tensor_copy(rden[32 * hh:32 * hh + 1, :],
                                              outUs[hh][D_HEAD:VA, :])
                    rrec = nrm_pool.tile([33, w], F32, tag="rrec",
                                         name=f"rrec{t}_{j}", bufs=2)
                    nc.vector.reciprocal(rrec[:], rden[:])
                    rcast = nrm_pool.tile([33, w], F32R, tag="rcast",
                                          name=f"rcast{t}_{j}", bufs=2)
                    with nc.allow_low_precision(reason="denom cast f32r"):
                        nc.vector.tensor_copy(rcast[:], rrec[:])
                    rc_rows = [(rcast, 0), (rcast, 32)]

                # stage 2 (PE bcast + DVE mul): deferred into the NEXT
                # chunk stream (possibly the next pair's) so the PE never
                # waits on the reciprocal chain
                def make_fin(tt, jj, sq00, ww, oUs, rc, ajb):
                    def fin():
                        psb = ps_s_pool.tile([P, 2 * ww], F32, tag="ps_s",
                                             name=f"psb{tt}_{jj}")
                        for hh in range(2):
                            tile_, row = rc[hh]
                            nc.tensor.matmul(psb[0:D_HEAD,
                                                 hh * ww:(hh + 1) * ww],
                                             ones_sb[row:row + 1, :],
                                             tile_[row:row + 1, :],
                                             start=True, stop=True,
                                             tile_position=(row - row % 32,
                                                            0))
                        for hh in range(2):
                            with nc.allow_low_precision(reason="outT bf16"):
                                nc.vector.tensor_mul(
                                    outT[tt][hh * D_HEAD:(hh + 1) * D_HEAD,
                                             sq00:sq00 + ww],
                                    oUs[hh][0:D_HEAD, :],
                                    psb[0:D_HEAD, hh * ww:(hh + 1) * ww])
                        if ajb is not None:
                            fillers.extend(ajb(sq00, ww))
                    return fin

                pending[0] = make_fin(t, j, sq0, w, outUs, rc_rows, after_jb)
            if flush and pending[0] is not None:
                fin = pending[0]
                pending[0] = None
                fin()
            while fi < len(fillers):
                fillers[fi]()
                fi += 1

        # ---- phase A scaffolding (xT, wqk, wv all loaded upfront) ----
        with tc.tile_pool(name="xTp", bufs=1) as xTp, \
             tc.tile_pool(name="wqkp", bufs=1) as wqkp, \
             tc.tile_pool(name="wvp", bufs=1) as wvp, \
             tc.tile_pool(name="psA", bufs=2, space="PSUM") as psA:
            xT_all = xTp.tile([P, KC * SEQ], X_DT, tag="xT", name="xT_all")
            wqkT_sb = [wqkp.tile([P, KC * P], X_DT, tag=f"wqkT{t}",
                                 name=f"wqkT{t}") for t in range(8)]
            wv_all = wvp.tile([P, KC * V_COLS], X_DT, tag="wv", name="wv_all")
            # wqk arrives host-pretiled as [t p, k c]; DMA the two col
            # tiles the first exp needs before everything else, then x
            # (the 4MB long pole), then wv, then the remaining tiles
            for t in (0, 4):
                nc.sync.dma_start(wqkT_sb[t][:], wqk[t * P:(t + 1) * P, :])
            half = (KC // 2) * P
            for h in range(2):
                nc.scalar.dma_start(
                    xT_all[:, h * 4 * SEQ:(h + 1) * 4 * SEQ].rearrange(
                        "p (k c) -> p k c", k=KC // 2),
                    xT[h * half:(h + 1) * half, :].rearrange(
                        "(k p) c -> p k c", p=P))
            nc.scalar.dma_start(
                wv_all[:].rearrange("p (k c) -> p k c", k=KC),
                wv[:].rearrange("(k p) c -> p k c", p=P))
            for t in (1, 5, 2, 6, 3, 7):
                nc.scalar.dma_start(wqkT_sb[t][:], wqk[t * P:(t + 1) * P, :])
            xT_sb = [xT_all[:, k * SEQ:(k + 1) * SEQ] for k in range(KC)]
            wv_sb = [wv_all[:, k * V_COLS:(k + 1) * V_COLS]
                     for k in range(KC)]

            def make_qk_tile(t):
                if t < 4:
                    qT[t] = qkT_pool.tile([P, SEQ], QK_DT, tag="qTa",
                                          name=f"qT{t}")
                else:
                    kT[t - 4] = qkT_pool.tile([P, SEQ], QK_DT, tag="kTa",
                                              name=f"kT{t-4}")

            def qk_mm(t, j, k, state):
                """One matmul of column tile t (q if t<4 else k), sq/sk
                group j, contraction chunk k; bias-add evacuation at k=7."""
                dst = qT[t] if t < 4 else kT[t - 4]

                def go():
                    if k == 0:
                        state[j] = psA.tile([P, 512], F32, tag="psA_t",
                                            name=f"psqk{t}_{j}")
                    ps = state[j]
                    nc.tensor.matmul(
                        ps[:], wqkT_sb[t][:, k * P:(k + 1) * P],
                        xT_sb[k][:, j * 512:(j + 1) * 512],
                        start=(k == 0), stop=(k == KC - 1))
                    if k == KC - 1:
                        with nc.allow_low_precision(reason="qk bf16"):
                            nc.vector.tensor_scalar_add(
                                dst[:, j * 512:(j + 1) * 512], ps[:],
                                bqk_sb[:, t:t + 1])
                return go

            def v_mm(s, k, state):
                def go():
                    if k == 0:
                        state[s] = psA.tile([P, V_COLS], F32, tag="psA_t",
                                            name=f"psv{s}")
                    ps = state[s]
                    nc.tensor.matmul(
                        ps[:], xT_sb[k][:, s * P:(s + 1) * P], wv_sb[k][:],
                        start=(k == 0), stop=(k == KC - 1))
                    if k == KC - 1:
                        v3 = vsb[s][:].rearrange("p (h c) -> p h c", c=VA)
                        ps3 = ps[:].rearrange("p (h c) -> p h c", c=D_HEAD)
                        bv3 = bv_sb[:].rearrange("p (h c) -> p h c", c=D_HEAD)
                        with nc.allow_low_precision(reason="v bf16"):
                            nc.vector.tensor_add(v3[:, :, 0:D_HEAD], ps3, bv3)
                            nc.vector.tensor_scalar(
                                v3[:, :, D_HEAD], bv_sb[:, 0:N_HEADS_CORE],
                                0.0, 1.0, mybir.AluOpType.mult,
                                mybir.AluOpType.add)
                return go

            # minimal prologue: q jb0, k group0, v chunk0 — then attention
            # starts and everything else dribbles into the chunk stream
            JIT = os.environ.get("K_JIT", "1") == "1"
            make_qk_tile(0)
            make_qk_tile(4)
            st_q0, st_k0, st_v = {}, {}, {}
            if JIT:
                for k in range(KC):
                    qk_mm(0, 0, k, st_q0)()
                    qk_mm(4, 0, k, st_k0)()
            else:
                for j in range(NJB):
                    for k in range(KC):
                        qk_mm(0, j, k, st_q0)()
                for g in range(NJB):
                    for k in range(KC):
                        qk_mm(4, g, k, st_k0)()
                for s in range(NSC):
                    for k in range(KC):
                        v_mm(s, k, st_v)()

            # pair 0 fillers with need-by markers (JIT production):
            #   k^T group g   -> before iteration 4g-1 (scores of chunk 4g)
            #   v chunk s     -> before iteration s    (PV of chunk s)
            #   q^T block jq  -> before iteration 16jq-2 (scores of jb jq)
            f0 = []
            if JIT:
                for k in range(KC):
                    f0.append((0, v_mm(0, k, st_v)))
                for g in range(1, NJB):
                    for k in range(KC):
                        f0.append((4 * g - 2, qk_mm(4, g, k, st_k0)))
                for s in range(1, NSC):
                    for k in range(KC):
                        f0.append((s, v_mm(s, k, st_v)))
                for jq in range(1, NJB):
                    for k in range(KC):
                        f0.append((16 * jq - 3, qk_mm(0, jq, k, st_q0)))
            # pair 1's q/k production spread over pair 0's later chunks
            make_qk_tile(1)
            make_qk_tile(5)
            st1 = {"q": {}, "k": {}}
            p1 = [qk_mm(1, j, k, st1["q"]) for j in range(NJB)
                  for k in range(KC)]
            p1 += [qk_mm(5, j, k, st1["k"]) for j in range(NJB)
                   for k in range(KC)]
            for i, th in enumerate(p1):
                f0.append((34 + (28 * i) // len(p1), th))
            f0.sort(key=lambda x: x[0])
            emit_B_pair(0, [th for _, th in f0], psA,
                        need=[n for n, _ in f0], budget=3)

            # pairs 1-2, with pair p+1's q/k production dribbled in
            for pair in (1, 2):
                make_qk_tile(pair + 1)
                make_qk_tile(pair + 5)
                st = {"q": {}, "k": {}}
                fl = [qk_mm(pair + 1, j, k, st["q"]) for j in range(NJB)
                      for k in range(KC)]
                fl += [qk_mm(pair + 5, j, k, st["k"]) for j in range(NJB)
                       for k in range(KC)]
                emit_B_pair(pair, fl, psA)

        # ---- pair 3 + projection (xT/wqk/wv freed; wo loads there) ----
        with tc.tile_pool(name="wop", bufs=1) as wop, \
             tc.tile_pool(name="osb", bufs=4) as osb_pool, \
             tc.tile_pool(name="psC", bufs=2, space="PSUM") as psC:
            wo_sb = [wop.tile([P, EMB], X_DT, tag=f"wo{t}", name=f"wo{t}")
                     for t in range(4)]
            for t in range(4):
                nc.sync.dma_start(wo_sb[t][:], wo[t * P:(t + 1) * P, :])

            cstate = {}

            def one_c_mm(s, y, t):
                def go():
                    if t == 0:
                        cstate[(s, y)] = psC.tile([P, 512], F32, tag="psC_t",
                                                  name=f"psc{s}_{y}")
                    ps = cstate[(s, y)]
                    nc.tensor.matmul(
                        ps[:],
                        outT[t][:, s * P:(s + 1) * P],
                        wo_sb[t][:, y * 512:(y + 1) * 512],
                        start=(t == 0), stop=(t == 3))
                    if t == 3:
                        ot = osb_pool.tile([P, 512], BF16, tag="osb",
                                           name=f"osb{s}_{y}")
                        with nc.allow_low_precision(reason="out bf16"):
                            nc.vector.tensor_copy(ot[:], ps[:])
                        nc.sync.dma_start(
                            out[s * P:(s + 1) * P, y * 512:(y + 1) * 512],
                            ot[:])
                return go

            def emit_C_blk(sq0, w):
                return [one_c_mm(s, y, t)
                        for s in range(sq0 // P, (sq0 + w) // P)
                        for y in range(EMB // 512)
                        for t in range(4)]

            blks = None
            if os.environ.get("K_SPLIT", "0") == "1":
                blks = [(0, 512), (512, 512), (1024, 512),
                        (1536, 256), (1792, 256)]
            emit_B_pair(3, [], psC, after_jb=emit_C_blk, dynamic=True,
                        budget=2, flush=True, blocks=blks)

    nc.compile()
    return nc


def get_nc():
    global _CACHED
    if _CACHED is None:
        _CACHED = _build()
    return _CACHED


def make_in_maps(x, W_qkv, b_qkv, W_out, b_out):
    x = np.asarray(x, dtype=np.float32)
    W_qkv = np.asarray(W_qkv, dtype=np.float32)
    b_qkv = np.asarray(b_qkv, dtype=np.float32)
    W_out = np.asarray(W_out, dtype=np.float32)

    import os as _os
    BF = ml_dtypes.bfloat16 if _os.environ.get('K_X', 'bf16') == 'bf16' else np.float32
    in_maps = []
    for core in range(N_CORES):
        b, g = divmod(core, 2)
        c0 = g * 512
        wq = W_qkv[:, c0:c0 + 512] * NORM
        wk = W_qkv[:, EMB + c0:EMB + c0 + 512]
        wv_ = W_qkv[:, 2 * EMB + c0:2 * EMB + c0 + 512]
        bq = b_qkv[c0:c0 + 512] * NORM
        bk = b_qkv[EMB + c0:EMB + c0 + 512]
        bv_ = b_qkv[2 * EMB + c0:2 * EMB + c0 + 512]
        in_maps.append({
            "xT": np.ascontiguousarray(x[b].T).astype(BF),
            "wqk": np.ascontiguousarray(
                np.concatenate([wq, wk], axis=1).reshape(8, P, 8, P)
                .transpose(2, 1, 0, 3).reshape(EMB, QK_COLS)).astype(BF),
            "wv": np.ascontiguousarray(wv_).astype(BF),
            "wo": np.ascontiguousarray(W_out[c0:c0 + 512, :]).astype(BF),
            "bqk": np.ascontiguousarray(
                np.concatenate([bq, bk]).reshape(QK_COLS // P, P).T),
            "bv": bv_.reshape(1, V_COLS).astype(np.float32),
        })
    return in_maps


def kernel(x, W_qkv, b_qkv, W_out, b_out):
    nc = get_nc()
    b_out = np.asarray(b_out, dtype=np.float32)
    in_maps = make_in_maps(x, W_qkv, b_qkv, W_out, b_out)
    res = bass_utils.run_bass_kernel_spmd(nc, in_maps, core_ids=list(range(N_CORES)))
    outp = np.empty((4, SEQ, EMB), dtype=np.float32)
    for b in range(4):
        outp[b] = (res.results[2 * b]["out"].astype(np.float32)
                   + res.results[2 * b + 1]["out"].astype(np.float32)
                   + b_out)
    return outp
